# revision 29
# baseline (speedup 1.0000x reference)
"""Trainium2 Bass kernel for a video-diffusion BasicTransformerBlock
(sparse-causal self-attn + cross-attn + GEGLU FF).

Sharding: data-parallel, one (batch, frame) pair per NeuronCore (8 frames ->
8 cores). Each core receives its own frame, frame 0 of its batch, and the
previous frame (duplicated inputs), so the sparse-causal KV gather needs no
collectives. For frames 0/1 the first/former KV frames coincide; softmax over
duplicated keys is mathematically identical to the reference's concat.

On-device layout: activations are feature-major (x^T, [dim, tokens]) so every
projection contracts over SBUF partitions without any transposes. LayerNorm
column-stats come from ones-matmuls; softmax runs max-free (scores are
bounded ~|5.5|) with denominators from an appended ones-column in V.
All transposes happen host-side in numpy.

Numerics: the residual stream and its LN stats stay fp32r; LN outputs and
all projection weights are bf16 (dtype-matched matmuls); the first/former
KV frames are loaded, normalized, and projected entirely in bf16.
rstd is a single ACT Rsqrt (the Ln/Exp pair thrashed activation tables at
1.3us per flip); self-attention denominators use batched DVE reciprocals
(4 rows per 32-aligned partition); cross-attention denominators are a
batched ACT Reciprocal delayed past the exp stream so its table is swapped
exactly once.

v2 vs the 668us baseline: single-descriptor weight/input DMAs, k1/v1 loaded
once, cross-attn K/V built before self-attention (off phase D's critical
path), out-proj weights prefetched, ff1 weight DMAs issued up front, 3-deep
score PSUM pipeline, and the table-load fixes above.
"""
import os
import sys
import numpy as np

if not os.environ.get("TRN_TERMINAL_POOL_IPS"):
    raise RuntimeError("expected axon trn environment")
for _p in ("/opt/trn_rl_repo",):
    if _p not in sys.path:
        sys.path.append(_p)

import ml_dtypes
import concourse.bass as bass
import concourse.tile as tile
from concourse import bacc, mybir
from concourse.bass_utils import run_bass_kernel_spmd

FP32 = mybir.dt.float32
F32R = mybir.dt.float32r
BF16 = mybir.dt.bfloat16
AF = mybir.ActivationFunctionType
OP = mybir.AluOpType

D = 640          # model dim
T = 1024         # tokens / frame
H = 8            # heads
DH = 80          # head dim
DKT = D // 128   # 5 feature tiles of the model dim
TT = T // 128    # 8 token tiles / frame
QH = 512         # query half width
CROSS = 768
CKT = CROSS // 128
CTX = 77
CTXP = 80   # context padded for free-dim alignment
DFF = 2560       # ff hidden (per GEGLU half)
FMT = DFF // 128  # 20 ff row tiles per half
LN_EPS = 1e-5
VSLOT = 97       # per-head V slot width; ones col at 96

# bias-pack column offsets ([128, NB] f32)
OB1, OB2, FB2, FBX, FBG = 0, 5, 10, 15, 35
LN_G = {1: 55, 2: 65, 3: 75}
LN_B = {1: 60, 2: 70, 3: 80}
EPS_COL = 85
NB = 86

N_CORES = 8

# test hook: CoreSim lacks Gelu; tests may override with a sim-supported func
GELU_AF = None


def build_program(ln_trivial):
    nc = bacc.Bacc("TRN2", target_bir_lowering=False, debug=False,
                   num_devices=N_CORES)
    dram = {}
    dram["hsT_q"] = nc.dram_tensor("hsT_q", [D, T], F32R, kind="ExternalInput").ap()
    for name in ("hsT_first", "hsT_former"):
        dram[name] = nc.dram_tensor(name, [D, T], BF16, kind="ExternalInput").ap()
    dram["encT"] = nc.dram_tensor("encT", [CROSS, CTXP], BF16, kind="ExternalInput").ap()
    for name in ("q1", "k1", "v1", "q2", "o1", "o2"):
        dram[name] = nc.dram_tensor(name, [D, D], BF16, kind="ExternalInput").ap()
    for name in ("k2", "v2"):
        dram[name] = nc.dram_tensor(name, [CROSS, D], BF16, kind="ExternalInput").ap()
    dram["ff1b"] = nc.dram_tensor("ff1b", [2 * FMT, D, 128], BF16, kind="ExternalInput").ap()
    dram["ff2"] = nc.dram_tensor("ff2", [DFF, D], BF16, kind="ExternalInput").ap()
    dram["biases"] = nc.dram_tensor("biases", [128, NB], FP32, kind="ExternalInput").ap()
    out_dram = nc.dram_tensor("outT", [D, T], F32R, kind="ExternalOutput").ap()

    scale = float(DH) ** -0.5

    with tile.TileContext(nc) as tc:
        from contextlib import ExitStack
        with ExitStack() as ctx:
            pc = ctx.enter_context(tc.tile_pool(name="const", bufs=1))
            pres = ctx.enter_context(tc.tile_pool(name="res", bufs=1))
            pn = ctx.enter_context(tc.tile_pool(name="n", bufs=5))
            psq = ctx.enter_context(tc.tile_pool(name="sq", bufs=2))
            prow = ctx.enter_context(tc.tile_pool(name="row", bufs=1))
            prcb = ctx.enter_context(tc.tile_pool(name="rcb", bufs=2))
            pw = ctx.enter_context(tc.tile_pool(name="w", bufs=2))
            pps = ctx.enter_context(tc.tile_pool(name="ps", bufs=2, space="PSUM"))

            bias_sb = pc.tile([128, NB], FP32, tag="bias")
            nc.sync.dma_start(bias_sb[:], dram["biases"][:])
            invd_f = pc.tile([128, 1], FP32, tag="invdf")
            nc.vector.memset(invd_f[:], 1.0 / D)
            invd = pc.tile([128, 1], F32R, tag="invd")
            nc.vector.tensor_copy(invd[:], invd_f[:])  # fp32r rounding producer
            invd_b = pc.tile([128, 1], BF16, tag="invdb")
            nc.vector.tensor_copy(invd_b[:], invd_f[:])
            onesr_f = pc.tile([128, 128], FP32, tag="onesrf")
            nc.vector.memset(onesr_f[:], 1.0)
            onesr = pc.tile([128, 128], F32R, tag="onesr")
            nc.vector.tensor_copy(onesr[:], onesr_f[:])

            def bcol(j):
                return bias_sb[:, j:j + 1]

            def raw_act(out, in_, func, bias=0.0, scale=1.0, alpha=0.0):
                """InstActivation without bass's Rsqrt/Reciprocal lockout.

                The ACT spline tables for rsqrt/reciprocal are coarser than
                Ln+Exp round trips, but the Ln/Exp pair thrashes activation
                tables (1.28us per flip: bacc greedily picks the exp-less
                natural_log set for Ln). rstd/denominator accuracy here is
                validated end-to-end against the fp32 reference."""
                sb = nc.scalar
                ins = [sb.lower_ap(in_)]
                for arg in (bias, scale, alpha):
                    if isinstance(arg, bass.AP):
                        ins.append(sb.lower_ap(arg))
                    else:
                        ins.append(mybir.ImmediateValue(
                            dtype=mybir.dt.float32, value=float(arg)))
                return sb.add_instruction(
                    mybir.InstActivation(
                        name=sb.bass.get_next_instruction_name(),
                        func=func, ins=ins, outs=[sb.lower_ap(out)]))

            def load_w_big(dname, n_kt, width, tag, pool, bufs=2):
                """One [128, n_kt*width] bf16 tile per weight, single DMA
                descriptor; slice chunk kt at cols [kt*width, (kt+1)*width)."""
                wt = pool.tile([128, n_kt * width], BF16, tag=tag, name=dname,
                               bufs=bufs)
                dst = wt[:].rearrange("p (k c) -> p k c", c=width)
                src = dram[dname].rearrange("(k p) c -> p k c", p=128)
                nc.sync.dma_start(dst, src)
                return wt

            def wsl(wt, kt, width, c0, c1):
                return wt[:, kt * width + c0:kt * width + c1]

            def emit_ln(x_tiles, which, out_tag=None):
                """Feature-major LN of 5 [128, T] tiles.

                out_tag given: x is fp32r, results go to 5 new bf16 tiles.
                out_tag None: x is bf16 and the LN runs in place (used for
                the first/former KV frames, which live entirely in bf16).
                Column stats via ones-matmuls; mean/rstd rows for the two
                query halves are packed at partitions 0/32; broadcasting
                across partitions is a PE ones-column outer product into
                PSUM. rstd = Rsqrt(var+eps) in one ACT op (the Ln/Exp pair
                cost an activation-table flip on every call)."""
                in_place = out_tag is None
                ivd = invd_b if in_place else invd
                sqdt = BF16 if in_place else F32R
                out_tiles = x_tiles if in_place else []
                mup = prow.tile([128, QH], F32R, tag="mup", bufs=1, name=f"mup{which}")
                msqp = prow.tile([128, QH], FP32, tag="msqp", bufs=1, name=f"msqp{which}")
                rstd = prow.tile([128, QH], F32R, tag="rstd", bufs=1, name=f"rstd{which}")
                mu_b = {}
                for hh in range(2):
                    sl = slice(hh * QH, (hh + 1) * QH)
                    r0 = 32 * hh
                    stp = pps.tile([128, 2 * QH], FP32, tag="sps", bufs=3,
                                   name=f"lnps{which}{hh}")
                    sp = stp[:, 0:QH]
                    spq = stp[:, QH:2 * QH]
                    for kt in range(DKT):
                        nc.tensor.matmul(sp[0:1, :], ivd[:, 0:1],
                                         x_tiles[kt][:, sl],
                                         start=(kt == 0), stop=(kt == DKT - 1))
                    for kt in range(DKT):
                        sq = psq.tile([128, QH], sqdt, tag=f"sq{in_place}",
                                      name=f"sq{which}{hh}{kt}")
                        nc.scalar.square(sq[:], x_tiles[kt][:, sl])
                        nc.tensor.matmul(spq[0:1, :], ivd[:, 0:1], sq[:],
                                         start=(kt == 0), stop=(kt == DKT - 1))
                    nc.vector.tensor_copy(mup[r0:r0 + 1, :], sp[0:1, :])
                    nc.vector.tensor_copy(msqp[r0:r0 + 1, :], spq[0:1, :])
                    mb = pps.tile([128, QH], FP32, tag="avps", bufs=2,
                                  name=f"mub{which}{hh}")
                    nc.tensor.matmul(mb[:, :], onesr[r0:r0 + 1, :],
                                     mup[r0:r0 + 1, :], start=True, stop=True)
                    mu_b[hh] = mb
                    # pass 1: x - mu (frees the mu broadcast PSUM bank early)
                    for kt in range(DKT):
                        if in_place:
                            nt_seg = x_tiles[kt][:, sl]
                        else:
                            if hh == 0:
                                nt = pn.tile([128, T], BF16, tag=out_tag, bufs=5,
                                             name=f"n{which}_{kt}")
                                out_tiles.append(nt)
                            nt_seg = out_tiles[kt][:, sl]
                        nc.vector.tensor_tensor(nt_seg, x_tiles[kt][:, sl],
                                                mu_b[hh][:, :], OP.subtract)
                    # -var = mu^2 - E[x^2] at the packed row
                    nc.vector.tensor_tensor(mup[r0:r0 + 1, :], mup[r0:r0 + 1, :],
                                            mup[r0:r0 + 1, :], OP.mult)
                    nc.vector.tensor_tensor(mup[r0:r0 + 1, :], mup[r0:r0 + 1, :],
                                            msqp[r0:r0 + 1, :], OP.subtract)
                    # rstd = rsqrt(var + eps) in one ACT op
                    raw_act(rstd[r0:r0 + 1, :], mup[r0:r0 + 1, :],
                            AF.Rsqrt, scale=-1.0,
                            bias=bias_sb[0:1, EPS_COL:EPS_COL + 1])
                for hh in range(2):
                    sl = slice(hh * QH, (hh + 1) * QH)
                    r0 = 32 * hh
                    rb = pps.tile([128, QH], FP32, tag="avps", bufs=2,
                                  name=f"rb{which}{hh}")
                    nc.tensor.matmul(rb[:, :], onesr[r0:r0 + 1, :],
                                     rstd[r0:r0 + 1, :], start=True, stop=True)
                    for kt in range(DKT):
                        nt_seg = out_tiles[kt][:, sl]
                        nc.vector.tensor_tensor(nt_seg, nt_seg, rb[:, :], OP.mult)
                        if not ln_trivial[which - 1]:
                            nc.scalar.activation(nt_seg, nt_seg, AF.Identity,
                                                 bias=bcol(LN_B[which] + kt),
                                                 scale=bcol(LN_G[which] + kt))
                return out_tiles

            def dense_proj(w_big, n_tiles, n_kt, tag):
                """Dense out^T: 5 x [128, T] bf16 tiles via full-height
                matmuls (the per-head-padded layout wastes 37.5% of the PE
                array on 80-row outputs; here every matmul fills 128 rows
                and the head split happens via SBUF->SBUF DMA in repart)."""
                outs = []
                for c in range(DKT):
                    qp = pps.tile([128, 2 * QH], FP32, tag="sps", bufs=3,
                                  name=f"dp{tag}{c}")
                    for hh in range(2):
                        for kt in range(n_kt):
                            nc.tensor.matmul(
                                qp[:, hh * QH:(hh + 1) * QH],
                                wsl(w_big, kt, D, c * 128, (c + 1) * 128),
                                n_tiles[kt][:, hh * QH:(hh + 1) * QH],
                                start=(kt == 0), stop=(kt == n_kt - 1))
                    qd = pqd.tile([128, T], BF16, tag="qd", bufs=3,
                                  name=f"qd{tag}{c}")
                    nc.vector.tensor_copy(qd[:], qp[:, :])
                    outs.append(qd)
                return outs

            def repart(dense_tiles, dst_tiles, col_off):
                """dense feature rows 80h:80h+80 -> dst_tiles[h][0:80]
                (per-head padded layout) via partition-crossing SBUF DMA."""
                for h in range(H):
                    c0, r0 = divmod(DH * h, 128)
                    n0 = min(128 - r0, DH)
                    nc.sync.dma_start(dst_tiles[h][0:n0, col_off:col_off + T],
                                      dense_tiles[c0][r0:r0 + n0, :])
                    if n0 < DH:
                        nc.sync.dma_start(
                            dst_tiles[h][n0:DH, col_off:col_off + T],
                            dense_tiles[c0 + 1][0:DH - n0, :])

            def repart_rev(aT_t, tag):
                """per-head attention out [0:80] -> 5 dense [128, T] tiles so
                the out-projection contracts 640 rows instead of 1024 padded."""
                ad = [pad.tile([128, T], BF16, tag="ad", bufs=5,
                               name=f"ad{tag}{c}") for c in range(DKT)]
                for h in range(H):
                    c0, r0 = divmod(DH * h, 128)
                    n0 = min(128 - r0, DH)
                    nc.sync.dma_start(ad[c0][r0:r0 + n0, :], aT_t[h][0:n0, :])
                    if n0 < DH:
                        nc.sync.dma_start(ad[c0 + 1][0:DH - n0, :],
                                          aT_t[h][n0:DH, :])
                return ad

            def v_proj(n_tiles, vt, n_kt, w_big, n_tok, tok_off):
                """token-major V tile, per-head 97-col slots: data cols 0:80,
                ones col at 96 so the AV denominator lands on PSUM partition
                96 (engine APs must start at partition 0/32/64/96)."""
                slots = vt[:, 0:H * VSLOT].rearrange("p (h c) -> p h c", c=VSLOT)
                nc.gpsimd.memset(slots[:, :, 80:96], 0.0)
                nc.gpsimd.memset(slots[:, :, 96:97], 1.0)
                vpp = pps.tile([128, 2 * QH], FP32, tag="sps", bufs=3, name="vpp")
                for half in range(2):
                    vp = vpp[:, half * QH:half * QH + 320]
                    for kt in range(n_kt):
                        nc.tensor.matmul(
                            vp[0:n_tok, :],
                            n_tiles[kt][:, tok_off:tok_off + n_tok],
                            wsl(w_big, kt, D, half * 320, (half + 1) * 320),
                            start=(kt == 0), stop=(kt == n_kt - 1))
                    dst = vt[:, half * 4 * VSLOT:(half + 1) * 4 * VSLOT].rearrange(
                        "p (h c) -> p h c", c=VSLOT)[0:n_tok, :, 0:80]
                    src = vp[0:n_tok, :].rearrange("p (h c) -> p h c", c=80)
                    nc.vector.tensor_copy(dst, src)

            def attention(qT_t, kT_t, v_t, n_keytiles, key_dim_last, aT_t, e_pool,
                          recip_on_act=False, delay_normalize=False):
                """S^T -> exp -> AV; attention output is evicted unnormalized
                and the 16 per-(head, q-half) denominators are batched into
                four 32-row-aligned tiles so just four reciprocals run.
                recip_on_act uses one batched ACT Reciprocal per tile instead
                of the DVE divide pipeline; delay_normalize postpones all
                normalizes past the last exp so the ACT table is swapped
                exactly once."""
                den_t = {}
                denr_t = {}
                GRP = 3  # matmul APs may only start at partition 0/32/64

                def dslot(p):
                    return p // GRP, 32 * (p % GRP)

                def emit_group_normalize(t):
                    """reciprocal of den tile t + normalize its pairs."""
                    dr = prcb.tile([128, QH], F32R, tag="denr", bufs=2,
                                   name=f"denr{t}")
                    if recip_on_act:
                        raw_act(dr[:], den_t[t][:], AF.Reciprocal)
                    else:
                        with nc.allow_low_precision(reason="fp32r denom rounding"):
                            nc.vector.reciprocal(dr[:], den_t[t][:])
                    denr_t[t] = dr
                    # one shared rcb tile per group: reallocating per pair
                    # would grab the in-flight avp buffer of the lookahead
                    # combo (WAR deadlock on the PE queue)
                    rcb = pps.tile([128, QH], FP32, tag="avps", bufs=2,
                                   name=f"rcb{t}")
                    for p in range(GRP * t, min(GRP * t + GRP, n_pairs)):
                        h, hh = p // 2, p % 2
                        _, drow = dslot(p)
                        nc.tensor.matmul(
                            rcb[0:DH, :], onesr[drow:drow + 1, 0:DH],
                            dr[drow:drow + 1, :], start=True, stop=True)
                        seg = aT_t[h][0:DH, hh * QH:(hh + 1) * QH]
                        nc.vector.tensor_tensor(seg, seg, rcb[0:DH, :], OP.mult)
                npairs = (n_keytiles + 1) // 2
                n_pairs = 2 * H
                st = {}

                def emit_scores(p, pt):
                    h, hh = p // 2, p % 2
                    kts = [kt for kt in (2 * pt, 2 * pt + 1) if kt < n_keytiles]
                    spp = pps.tile([128, 2 * QH], FP32, tag="sps",
                                   bufs=3, name=f"s{h}{hh}{pt}")
                    klens = []
                    for j, kt in enumerate(kts):
                        klen = key_dim_last if kt == n_keytiles - 1 else 128
                        klens.append(klen)
                        nc.tensor.matmul(
                            spp[0:klen, j * QH:(j + 1) * QH],
                            kT_t[h][0:DH, kt * 128:kt * 128 + klen],
                            qT_t[h][0:DH, hh * QH:(hh + 1) * QH],
                            start=True, stop=True)
                    et = e_pool.tile([128, 2 * QH], BF16, tag="E",
                                     name=f"e{h}{hh}{pt}")
                    if len(kts) == 2 and klens[0] == klens[1]:
                        nc.scalar.activation(et[0:klens[0], :],
                                             spp[0:klens[0], :],
                                             AF.Exp, scale=scale)
                    else:
                        for j, kt in enumerate(kts):
                            nc.scalar.activation(
                                et[0:klens[j], j * QH:(j + 1) * QH],
                                spp[0:klens[j], j * QH:(j + 1) * QH],
                                AF.Exp, scale=scale)
                    st[p]["ets"][pt] = (et, kts, klens)

                def emit_av(p, pt):
                    h, hh = p // 2, p % 2
                    avp = st[p]["avp"]
                    pet, pkts, pklens = st[p]["ets"].pop(pt)
                    for j, kt in enumerate(pkts):
                        nc.tensor.matmul(
                            avp[0:VSLOT, :],
                            v_t[kt][0:pklens[j], h * VSLOT:(h + 1) * VSLOT],
                            pet[0:pklens[j], j * QH:(j + 1) * QH],
                            start=(kt == 0), stop=(kt == n_keytiles - 1))

                def finish_combo(p):
                    """AV tail + unnormalized evict + denominator stash."""
                    h, hh = p // 2, p % 2
                    emit_av(p, npairs - 1)
                    avp = st.pop(p)["avp"]
                    nc.vector.tensor_copy(
                        aT_t[h][0:DH, hh * QH:(hh + 1) * QH], avp[0:DH, :])
                    dt_i, drow = dslot(p)
                    if dt_i not in den_t:
                        dn = prcb.tile([128, QH], BF16, tag="den", bufs=6,
                                       name=f"den{dt_i}")
                        nc.gpsimd.memset(dn[:], 1.0)
                        den_t[dt_i] = dn
                    nc.vector.tensor_copy(
                        den_t[dt_i][drow:drow + 1, :], avp[96:97, :])
                    if not delay_normalize and (
                            p == GRP * dt_i + GRP - 1 or p == n_pairs - 1):
                        emit_group_normalize(dt_i)

                # software-pipelined across (head, q-half) combos: the next
                # combo's first score group is emitted before the previous
                # combo's AV tail, so the PE has work while ACT finishes the
                # last exp of the old combo
                for p in range(n_pairs):
                    st[p] = {"avp": pps.tile([128, QH], FP32, tag="avps",
                                             bufs=2, name=f"av{p}"),
                             "ets": {}}
                    for pt in range(npairs):
                        emit_scores(p, pt)
                        if pt == 0:
                            if p > 0:
                                finish_combo(p - 1)
                        else:
                            emit_av(p, pt - 1)
                finish_combo(n_pairs - 1)
                if delay_normalize:
                    for t in sorted(den_t):
                        emit_group_normalize(t)

            def out_proj(w_big, ad_t, res_t, bias_off):
                """res += a @ o + bias (in-place residual update), contracting
                5 dense feature tiles; token-half-major so the next LN's
                half-0 stats can start while half 1 is still projecting."""
                for hh in range(2):
                    sl = slice(hh * QH, (hh + 1) * QH)
                    for m in range(DKT):
                        op_ = pps.tile([128, QH], FP32, tag="avps", bufs=2,
                                       name=f"op{m}{hh}")
                        for kt in range(DKT):
                            nc.tensor.matmul(
                                op_[:, :],
                                wsl(w_big, kt, D, m * 128, (m + 1) * 128),
                                ad_t[kt][:, sl],
                                start=(kt == 0), stop=(kt == DKT - 1))
                        nc.vector.scalar_tensor_tensor(
                            res_t[m][:, sl], op_[:, :], bcol(bias_off + m),
                            res_t[m][:, sl], OP.add, OP.add)

            def load_frame(dname, tag, pool, dtype, bufs=1):
                """One [128, 5*T] tile per frame, single DMA descriptor."""
                ft = pool.tile([128, DKT * T], dtype, tag=tag, name=dname,
                               bufs=bufs)
                dst = ft[:].rearrange("p (k c) -> p k c", c=T)
                src = dram[dname].rearrange("(k p) c -> p k c", p=128)
                nc.sync.dma_start(dst, src)
                return [ft[:, kt * T:(kt + 1) * T] for kt in range(DKT)]

            # residual stream (feature-major, f32), one DMA descriptor
            res_tiles = load_frame("hsT_q", "res", pres, F32R)

            with ExitStack() as ctx_abcd:
                pqT = ctx_abcd.enter_context(tc.tile_pool(name="qT", bufs=8))
                paT = ctx_abcd.enter_context(tc.tile_pool(name="aT", bufs=8))
                penc = ctx_abcd.enter_context(tc.tile_pool(name="enc", bufs=1))
                pk2 = ctx_abcd.enter_context(tc.tile_pool(name="k2T", bufs=8))
                pV2 = ctx_abcd.enter_context(tc.tile_pool(name="V2", bufs=1))
                pwp = ctx_abcd.enter_context(tc.tile_pool(name="wp", bufs=1))
                pqd = ctx_abcd.enter_context(tc.tile_pool(name="qd", bufs=3))

                # ---------- phase A: LN1 + QKV projections ----------
                with ExitStack() as ctx_b:
                    pfr = ctx_b.enter_context(tc.tile_pool(name="fr", bufs=2))
                    pkT = ctx_b.enter_context(tc.tile_pool(name="kT", bufs=8))
                    pV = ctx_b.enter_context(tc.tile_pool(name="V", bufs=16))
                    pE = ctx_b.enter_context(tc.tile_pool(name="E", bufs=3))

                    kT_tiles = [pkT.tile([128, 2 * T], BF16, tag="kT", name=f"kT_{h}")
                                for h in range(H)]
                    v_tiles = [pV.tile([128, H * VSLOT], BF16, tag="V", name=f"v_{i}")
                               for i in range(2 * TT)]

                    n_q = emit_ln(res_tiles, 1, "n")
                    fr0_tiles = load_frame("hsT_first", "fr", pfr, BF16, bufs=2)
                    emit_ln(fr0_tiles, 1)  # in place, overlaps Q proj
                    q1_sb = load_w_big("q1", DKT, D, "w", pw)
                    # issue the cross-attn input/weight fetches early; their
                    # projections are emitted after the fi loop
                    enc_big = penc.tile([128, CKT * CTXP], BF16, tag="enc",
                                        name="enc")
                    enc_dst = enc_big[:].rearrange("p (k c) -> p k c", c=CTXP)
                    enc_src = dram["encT"].rearrange("(k p) c -> p k c", p=128)
                    nc.sync.dma_start(enc_dst, enc_src)
                    k2_sb = load_w_big("k2", CKT, D, "w6", pw, bufs=1)
                    qT_tiles = [pqT.tile([128, T], BF16, tag="qT", name=f"qT_{h}")
                                for h in range(H)]
                    repart(dense_proj(q1_sb, n_q, DKT, "q"), qT_tiles, 0)

                    k1_sb = load_w_big("k1", DKT, D, "w", pw)
                    v1_sb = load_w_big("v1", DKT, D, "w", pw)
                    fr1_tiles = load_frame("hsT_former", "fr", pfr, BF16, bufs=2)
                    emit_ln(fr1_tiles, 1)  # in place, overlaps K/V of frame 0
                    for fi, fr_n in enumerate((fr0_tiles, fr1_tiles)):
                        repart(dense_proj(k1_sb, fr_n, DKT, f"k{fi}"),
                               kT_tiles, fi * T)
                        for tt in range(TT):
                            v_proj(fr_n, v_tiles[fi * TT + tt], DKT, v1_sb,
                                   128, tt * 128)

                    # cross-attn K/V: no dependency on attn1 -- build early
                    enc_tiles = [enc_big[:, kt * CTXP:(kt + 1) * CTXP]
                                 for kt in range(CKT)]
                    k2T_tiles = [pk2.tile([128, CTXP], BF16, tag="k2T",
                                          name=f"k2T_{h}") for h in range(H)]
                    for h in range(H):
                        kp = pps.tile([128, CTXP], FP32, tag="avps", bufs=2,
                                      name=f"k2p{h}")
                        for kt in range(CKT):
                            nc.tensor.matmul(kp[0:DH, :],
                                             wsl(k2_sb, kt, D, h * DH, (h + 1) * DH),
                                             enc_tiles[kt],
                                             start=(kt == 0), stop=(kt == CKT - 1))
                        nc.vector.tensor_copy(k2T_tiles[h][0:DH, :], kp[0:DH, :])
                    v2_sb = load_w_big("v2", CKT, D, "w6", pw, bufs=1)
                    v2_t = pV2.tile([128, H * VSLOT], BF16, tag="V2", name="v2t")
                    v_proj(enc_tiles, v2_t, CKT, v2_sb, CTX, 0)
                    # prefetch the o1 out-proj weights behind the attention PE stream
                    o1_sb = load_w_big("o1", DKT, D, "wp", pwp, bufs=1)

                    # ---------- phase B: sparse-causal attention ----------
                    aT_tiles = [paT.tile([128, T], BF16, tag="aT", name=f"aT_{h}")
                                for h in range(H)]
                    attention(qT_tiles, kT_tiles, v_tiles, 2 * TT, 128, aT_tiles, pE)

                # ---------- phases C+D: o1 + residual, cross attention ----------
                with ExitStack() as ctx_d:
                    pad = ctx_d.enter_context(tc.tile_pool(name="ad", bufs=5))
                    pE2 = ctx_d.enter_context(tc.tile_pool(name="E2", bufs=3))

                    out_proj(o1_sb, repart_rev(aT_tiles, "a1"), res_tiles, OB1)

                    o2_sb = load_w_big("o2", DKT, D, "wp", pwp, bufs=1)
                    n2 = emit_ln(res_tiles, 2, "n")
                    q2_sb = load_w_big("q2", DKT, D, "w", pw)
                    q2T_tiles = [pqT.tile([128, T], BF16, tag="qT", name=f"q2T_{h}")
                                 for h in range(H)]
                    repart(dense_proj(q2_sb, n2, DKT, "q2"), q2T_tiles, 0)

                    a2T_tiles = [paT.tile([128, T], BF16, tag="aT", name=f"a2T_{h}")
                                 for h in range(H)]
                    attention(q2T_tiles, k2T_tiles, [v2_t], 1, CTX, a2T_tiles, pE2,
                              recip_on_act=True, delay_normalize=True)
                    out_proj(o2_sb, repart_rev(a2T_tiles, "a2"), res_tiles, OB2)

            # ---------- phase E: GEGLU feed-forward ----------
            with ExitStack() as ctx_e:
                pG = ctx_e.enter_context(tc.tile_pool(name="gT", bufs=20))
                pgl = ctx_e.enter_context(tc.tile_pool(name="gl", bufs=3))
                pff1 = ctx_e.enter_context(tc.tile_pool(name="ff1w", bufs=40))
                pff2 = ctx_e.enter_context(tc.tile_pool(name="ff2w", bufs=1))

                # issue every ff weight DMA up front so the fetch overlaps the
                # cross-attention tail instead of trickling in per row-tile
                fxg = []
                for mi in range(2 * FMT):
                    fw = pff1.tile([128, D], BF16, tag="ff1w", name=f"fw{mi}")
                    fw_dst = fw[:].rearrange("p (k c) -> p k c", c=128)
                    fw_src = dram["ff1b"][mi].rearrange("(k p) c -> p k c", p=128)
                    nc.sync.dma_start(fw_dst, fw_src)
                    fxg.append(fw)
                ff2_sb = load_w_big("ff2", FMT, D, "ff2w", pff2, bufs=1)

                n3 = emit_ln(res_tiles, 3, "n")
                # token-half-major: ff1+ff2 for half 0 only wait on LN3's
                # half-0 stats, overlapping the cross-attention tail
                gT_tiles = []
                for hh in range(2):
                    for mi in range(FMT):
                        fx, fg = fxg[mi], fxg[FMT + mi]
                        if hh == 0:
                            gT_tiles.append(pG.tile([128, T], BF16, tag="gT",
                                                    name=f"gT_{mi}"))
                        gt = gT_tiles[mi]
                        xgp = pps.tile([128, 2 * QH], FP32, tag="sps", bufs=3,
                                       name=f"xgp{mi}{hh}")
                        xp = xgp[:, 0:QH]
                        gp = xgp[:, QH:2 * QH]
                        for kt in range(DKT):
                            nc.tensor.matmul(
                                xp[:, :], fx[:, kt * 128:(kt + 1) * 128],
                                n3[kt][:, hh * QH:(hh + 1) * QH],
                                start=(kt == 0), stop=(kt == DKT - 1))
                        for kt in range(DKT):
                            nc.tensor.matmul(
                                gp[:, :], fg[:, kt * 128:(kt + 1) * 128],
                                n3[kt][:, hh * QH:(hh + 1) * QH],
                                start=(kt == 0), stop=(kt == DKT - 1))
                        gl = pgl.tile([128, QH], BF16, tag="gl", name=f"gl{mi}{hh}")
                        nc.scalar.activation(gl[:], gp[:, :], GELU_AF or AF.Gelu,
                                             bias=bcol(FBG + mi), scale=1.0)
                        nc.vector.scalar_tensor_tensor(
                            gt[:, hh * QH:(hh + 1) * QH], xp[:, :], bcol(FBX + mi),
                            gl[:], OP.add, OP.mult)

                    sl = slice(hh * QH, (hh + 1) * QH)
                    for m in range(DKT):
                        fp = pps.tile([128, QH], FP32, tag="avps", bufs=2,
                                      name=f"fp{m}{hh}")
                        for kt in range(FMT):
                            nc.tensor.matmul(
                                fp[:, :],
                                wsl(ff2_sb, kt, D, m * 128, (m + 1) * 128),
                                gT_tiles[kt][:, sl],
                                start=(kt == 0), stop=(kt == FMT - 1))
                        nc.vector.scalar_tensor_tensor(
                            res_tiles[m][:, sl], fp[:, :], bcol(FB2 + m),
                            res_tiles[m][:, sl], OP.add, OP.add)
                        # stream the finished [chunk, half] straight out
                        nc.sync.dma_start(
                            out_dram[m * 128:(m + 1) * 128, sl],
                            res_tiles[m][:, sl])

    nc.compile()
    return nc


def _install_ntff_shim():
    """Register the axon NTFF profile hook (profiling only; this container's
    antenv lacks the axon_hooks shim module)."""
    import types
    if "antenv.axon_hooks" in sys.modules:
        return
    mod = types.ModuleType("antenv.axon_hooks")
    mod._hook = None
    mod.set_axon_ntff_profile_hook = lambda h: setattr(mod, "_hook", h)
    mod.get_axon_ntff_profile_hook = lambda: mod._hook
    sys.modules["antenv.axon_hooks"] = mod
    try:
        from trn_agent_boot.trn_boot import _ntff_profile_via_ctypes
        mod._hook = _ntff_profile_via_ctypes("/opt/axon/libaxon_pjrt.so")
    except Exception:
        pass


_PROGRAM_CACHE = {}


def _get_program(ln_trivial):
    key = (tuple(ln_trivial), GELU_AF)
    if key not in _PROGRAM_CACHE:
        _PROGRAM_CACHE[key] = build_program(ln_trivial)
    return _PROGRAM_CACHE[key]


def _pad_heads(w):
    """[640, 640] head rows -> [1024, 640] padded to 128/head."""
    out = np.zeros((H * 128, D), np.float32)
    for h in range(H):
        out[h * 128:h * 128 + DH] = w[h * DH:(h + 1) * DH]
    return out


def _bias_cols(vec, n):
    return np.ascontiguousarray(vec.reshape(n, 128).T)


def _bf(a):
    return np.ascontiguousarray(np.asarray(a, np.float32)).astype(ml_dtypes.bfloat16)


def kernel(**inputs):
    hs = np.ascontiguousarray(inputs["hidden_states"], np.float32)
    enc = np.ascontiguousarray(inputs["encoder_hidden_states"], np.float32)
    f = int(inputs["video_length"])
    BF = hs.shape[0]
    assert BF == N_CORES and hs.shape[1:] == (T, D)

    ln_trivial = tuple(
        bool(np.all(inputs[f"n{i}_g"] == 1.0) and np.all(inputs[f"n{i}_b"] == 0.0))
        for i in (1, 2, 3))
    nc = _get_program(ln_trivial)

    biases = np.zeros((128, NB), np.float32)
    biases[:, EPS_COL] = LN_EPS
    biases[:, OB1:OB1 + 5] = _bias_cols(inputs["o1_b"].astype(np.float32), 5)
    biases[:, OB2:OB2 + 5] = _bias_cols(inputs["o2_b"].astype(np.float32), 5)
    biases[:, FB2:FB2 + 5] = _bias_cols(inputs["ff2_b"].astype(np.float32), 5)
    ff1_b = inputs["ff1_b"].astype(np.float32)
    biases[:, FBX:FBX + FMT] = _bias_cols(ff1_b[:DFF], FMT)
    biases[:, FBG:FBG + FMT] = _bias_cols(ff1_b[DFF:], FMT)
    for i in (1, 2, 3):
        biases[:, LN_G[i]:LN_G[i] + 5] = _bias_cols(inputs[f"n{i}_g"].astype(np.float32), 5)
        biases[:, LN_B[i]:LN_B[i] + 5] = _bias_cols(inputs[f"n{i}_b"].astype(np.float32), 5)

    ff1 = inputs["ff1"].astype(np.float32)  # [640, 5120]
    ff1b = np.ascontiguousarray(
        ff1.reshape(DKT, 128, 2 * FMT, 128).transpose(2, 0, 1, 3).reshape(2 * FMT, D, 128))

    common = {
        "q1": _bf(inputs["q1"]),
        "k1": _bf(inputs["k1"]),
        "v1": _bf(inputs["v1"]),
        "q2": _bf(inputs["q2"]),
        "k2": _bf(inputs["k2"]),
        "v2": _bf(inputs["v2"]),
        "o1": _bf(inputs["o1"]),
        "o2": _bf(inputs["o2"]),
        "ff1b": ff1b.astype(ml_dtypes.bfloat16),
        "ff2": _bf(inputs["ff2"]),
        "biases": biases,
    }

    hsT = np.ascontiguousarray(hs.transpose(0, 2, 1))      # [BF, 640, 1024]
    hsTb = hsT.astype(ml_dtypes.bfloat16)
    encT = np.zeros((BF, CROSS, CTXP), np.float32)         # ctx padded 77 -> 80
    encT[:, :, :CTX] = enc.transpose(0, 2, 1)
    encTb = encT.astype(ml_dtypes.bfloat16)
    in_maps = []
    for g in range(BF):
        bi, fi = divmod(g, f)
        first = bi * f
        former = bi * f + max(fi - 1, 0)
        in_maps.append({
            **common,
            "hsT_q": hsT[g],
            "hsT_first": hsTb[first],
            "hsT_former": hsTb[former],
            "encT": encTb[g],
        })

    want_trace = bool(int(os.environ.get("KERNEL_TRACE", "0")))
    if want_trace:
        _install_ntff_shim()
    res = run_bass_kernel_spmd(nc, in_maps, core_ids=list(range(N_CORES)),
                               trace=want_trace)
    kernel.last_results = res
    out = np.stack([res.results[g]["outT"].T for g in range(BF)])
    return np.ascontiguousarray(out.astype(inputs["hidden_states"].dtype))


# revision 34
# speedup vs baseline: 1.2048x; 1.2048x over previous
"""Trainium2 Bass kernel for a video-diffusion BasicTransformerBlock
(sparse-causal self-attn + cross-attn + GEGLU FF).

Sharding: data-parallel, one (batch, frame) pair per NeuronCore (8 frames ->
8 cores). Each core receives its own frame, frame 0 of its batch, and the
previous frame (duplicated inputs), so the sparse-causal KV gather needs no
collectives. For frames 0/1 the first/former KV frames coincide; softmax over
duplicated keys is mathematically identical to the reference's concat.

On-device layout: activations are feature-major (x^T, [dim, tokens]) so every
projection contracts over SBUF partitions without any transposes. LayerNorm
column-stats come from ones-matmuls; softmax runs max-free (scores are
bounded ~|5.5|) with denominators from an appended ones-column in V.
All transposes happen host-side in numpy.

Numerics: the residual stream and its LN stats stay fp32r; LN outputs and
all projection weights are bf16 (dtype-matched matmuls); the first/former
KV frames are loaded, normalized, and projected entirely in bf16.
rstd is a single ACT Rsqrt (the Ln/Exp pair thrashed activation tables at
1.3us per flip); self-attention denominators use batched DVE reciprocals
(3 rows per tile at partitions 0/32/64); cross-attention denominators are
batched ACT Reciprocals delayed past the exp stream so its table is swapped
exactly once.

vs the 668us baseline: Q/K/out projections run dense (full 128-row matmuls,
then SBUF->SBUF DMA repartition to/from the per-head padded layout instead
of 80-row matmuls that idle 37.5% of the PE), single-descriptor
weight/input DMAs, k1/v1 loaded once, cross-attn K/V built before
self-attention (off phase D's critical path), out-proj weights prefetched,
ff1 weight DMAs issued up front, ff/out-proj loops token-half-major so each
phase's half-0 consumer starts early, the finished residual streams out
per (chunk, half), and a 3-deep score PSUM pipeline. Measured 556.9us at
rel err 3.5e-3 (slowest of 8 cores, neuron-profile).
"""
import os
import sys
import numpy as np

if not os.environ.get("TRN_TERMINAL_POOL_IPS"):
    raise RuntimeError("expected axon trn environment")
for _p in ("/opt/trn_rl_repo",):
    if _p not in sys.path:
        sys.path.append(_p)

import ml_dtypes
import concourse.bass as bass
import concourse.tile as tile
from concourse import bacc, mybir
from concourse.bass_utils import run_bass_kernel_spmd

FP32 = mybir.dt.float32
F32R = mybir.dt.float32r
BF16 = mybir.dt.bfloat16
AF = mybir.ActivationFunctionType
OP = mybir.AluOpType

D = 640          # model dim
T = 1024         # tokens / frame
H = 8            # heads
DH = 80          # head dim
DKT = D // 128   # 5 feature tiles of the model dim
TT = T // 128    # 8 token tiles / frame
QH = 512         # query half width
CROSS = 768
CKT = CROSS // 128
CTX = 77
CTXP = 80   # context padded for free-dim alignment
DFF = 2560       # ff hidden (per GEGLU half)
FMT = DFF // 128  # 20 ff row tiles per half
LN_EPS = 1e-5
VSLOT = 97       # per-head V slot width; ones col at 96

# bias-pack column offsets ([128, NB] f32)
OB1, OB2, FB2, FBX, FBG = 0, 5, 10, 15, 35
LN_G = {1: 55, 2: 65, 3: 75}
LN_B = {1: 60, 2: 70, 3: 80}
EPS_COL = 85
NB = 86

N_CORES = 8

# test hook: CoreSim lacks Gelu; tests may override with a sim-supported func
GELU_AF = None


def build_program(ln_trivial):
    nc = bacc.Bacc("TRN2", target_bir_lowering=False, debug=False,
                   num_devices=N_CORES)
    dram = {}
    dram["hsT_q"] = nc.dram_tensor("hsT_q", [D, T], F32R, kind="ExternalInput").ap()
    for name in ("hsT_first", "hsT_former"):
        dram[name] = nc.dram_tensor(name, [D, T], BF16, kind="ExternalInput").ap()
    dram["encT"] = nc.dram_tensor("encT", [CROSS, CTXP], BF16, kind="ExternalInput").ap()
    for name in ("q1", "k1", "v1", "q2", "o1", "o2"):
        dram[name] = nc.dram_tensor(name, [D, D], BF16, kind="ExternalInput").ap()
    for name in ("k2", "v2"):
        dram[name] = nc.dram_tensor(name, [CROSS, D], BF16, kind="ExternalInput").ap()
    dram["ff1b"] = nc.dram_tensor("ff1b", [2 * FMT, D, 128], BF16, kind="ExternalInput").ap()
    dram["ff2"] = nc.dram_tensor("ff2", [DFF, D], BF16, kind="ExternalInput").ap()
    dram["biases"] = nc.dram_tensor("biases", [128, NB], FP32, kind="ExternalInput").ap()
    out_dram = nc.dram_tensor("outT", [D, T], F32R, kind="ExternalOutput").ap()

    scale = float(DH) ** -0.5

    with tile.TileContext(nc) as tc:
        from contextlib import ExitStack
        with ExitStack() as ctx:
            pc = ctx.enter_context(tc.tile_pool(name="const", bufs=1))
            pres = ctx.enter_context(tc.tile_pool(name="res", bufs=1))
            pn = ctx.enter_context(tc.tile_pool(name="n", bufs=5))
            psq = ctx.enter_context(tc.tile_pool(name="sq", bufs=2))
            prow = ctx.enter_context(tc.tile_pool(name="row", bufs=1))
            prcb = ctx.enter_context(tc.tile_pool(name="rcb", bufs=2))
            pw = ctx.enter_context(tc.tile_pool(name="w", bufs=2))
            pps = ctx.enter_context(tc.tile_pool(name="ps", bufs=2, space="PSUM"))

            bias_sb = pc.tile([128, NB], FP32, tag="bias")
            nc.sync.dma_start(bias_sb[:], dram["biases"][:])
            invd_f = pc.tile([128, 1], FP32, tag="invdf")
            nc.vector.memset(invd_f[:], 1.0 / D)
            invd = pc.tile([128, 1], F32R, tag="invd")
            nc.vector.tensor_copy(invd[:], invd_f[:])  # fp32r rounding producer
            invd_b = pc.tile([128, 1], BF16, tag="invdb")
            nc.vector.tensor_copy(invd_b[:], invd_f[:])
            onesr_f = pc.tile([128, 128], FP32, tag="onesrf")
            nc.vector.memset(onesr_f[:], 1.0)
            onesr = pc.tile([128, 128], F32R, tag="onesr")
            nc.vector.tensor_copy(onesr[:], onesr_f[:])

            # warm the PE HAM clock gate (cold = 1.2GHz, warm = 2.4GHz after
            # ~3.4us of sustained activity) with throwaway matmuls while the
            # input DMAs are still in flight
            warm = pps.tile([128, QH], FP32, tag="avps", bufs=2, name="warm")
            for _ in range(40):
                nc.tensor.matmul(warm[:, 0:128], onesr[:, :], onesr[:, :],
                                 start=True, stop=True)

            def bcol(j):
                return bias_sb[:, j:j + 1]

            def raw_act(out, in_, func, bias=0.0, scale=1.0, alpha=0.0):
                """InstActivation without bass's Rsqrt/Reciprocal lockout.

                The ACT spline tables for rsqrt/reciprocal are coarser than
                Ln+Exp round trips, but the Ln/Exp pair thrashes activation
                tables (1.28us per flip: bacc greedily picks the exp-less
                natural_log set for Ln). rstd/denominator accuracy here is
                validated end-to-end against the fp32 reference."""
                sb = nc.scalar
                ins = [sb.lower_ap(in_)]
                for arg in (bias, scale, alpha):
                    if isinstance(arg, bass.AP):
                        ins.append(sb.lower_ap(arg))
                    else:
                        ins.append(mybir.ImmediateValue(
                            dtype=mybir.dt.float32, value=float(arg)))
                return sb.add_instruction(
                    mybir.InstActivation(
                        name=sb.bass.get_next_instruction_name(),
                        func=func, ins=ins, outs=[sb.lower_ap(out)]))

            def load_w_big(dname, n_kt, width, tag, pool, bufs=2):
                """One [128, n_kt*width] bf16 tile per weight, single DMA
                descriptor; slice chunk kt at cols [kt*width, (kt+1)*width)."""
                wt = pool.tile([128, n_kt * width], BF16, tag=tag, name=dname,
                               bufs=bufs)
                dst = wt[:].rearrange("p (k c) -> p k c", c=width)
                src = dram[dname].rearrange("(k p) c -> p k c", p=128)
                nc.sync.dma_start(dst, src)
                return wt

            def wsl(wt, kt, width, c0, c1):
                return wt[:, kt * width + c0:kt * width + c1]

            def emit_ln(x_tiles, which, out_tag=None):
                """Feature-major LN of 5 [128, T] tiles.

                out_tag given: x is fp32r, results go to 5 new bf16 tiles.
                out_tag None: x is bf16 and the LN runs in place (used for
                the first/former KV frames, which live entirely in bf16).
                Column stats via ones-matmuls; mean/rstd rows for the two
                query halves are packed at partitions 0/32; broadcasting
                across partitions is a PE ones-column outer product into
                PSUM. rstd = Rsqrt(var+eps) in one ACT op (the Ln/Exp pair
                cost an activation-table flip on every call)."""
                in_place = out_tag is None
                ivd = invd_b if in_place else invd
                sqdt = BF16 if in_place else F32R
                out_tiles = x_tiles if in_place else []
                mup = prow.tile([128, QH], F32R, tag="mup", bufs=1, name=f"mup{which}")
                msqp = prow.tile([128, QH], FP32, tag="msqp", bufs=1, name=f"msqp{which}")
                rstd = prow.tile([128, QH], F32R, tag="rstd", bufs=1, name=f"rstd{which}")
                mu_b = {}
                for hh in range(2):
                    sl = slice(hh * QH, (hh + 1) * QH)
                    r0 = 32 * hh
                    stp = pps.tile([128, 2 * QH], FP32, tag="sps", bufs=3,
                                   name=f"lnps{which}{hh}")
                    sp = stp[:, 0:QH]
                    spq = stp[:, QH:2 * QH]
                    for kt in range(DKT):
                        nc.tensor.matmul(sp[0:1, :], ivd[:, 0:1],
                                         x_tiles[kt][:, sl],
                                         start=(kt == 0), stop=(kt == DKT - 1))
                    for kt in range(DKT):
                        sq = psq.tile([128, QH], sqdt, tag=f"sq{in_place}",
                                      name=f"sq{which}{hh}{kt}")
                        nc.scalar.square(sq[:], x_tiles[kt][:, sl])
                        nc.tensor.matmul(spq[0:1, :], ivd[:, 0:1], sq[:],
                                         start=(kt == 0), stop=(kt == DKT - 1))
                    nc.vector.tensor_copy(mup[r0:r0 + 1, :], sp[0:1, :])
                    nc.vector.tensor_copy(msqp[r0:r0 + 1, :], spq[0:1, :])
                    mb = pps.tile([128, QH], FP32, tag="avps", bufs=2,
                                  name=f"mub{which}{hh}")
                    nc.tensor.matmul(mb[:, :], onesr[r0:r0 + 1, :],
                                     mup[r0:r0 + 1, :], start=True, stop=True)
                    mu_b[hh] = mb
                    # pass 1: x - mu (frees the mu broadcast PSUM bank early)
                    for kt in range(DKT):
                        if in_place:
                            nt_seg = x_tiles[kt][:, sl]
                        else:
                            if hh == 0:
                                nt = pn.tile([128, T], BF16, tag=out_tag, bufs=5,
                                             name=f"n{which}_{kt}")
                                out_tiles.append(nt)
                            nt_seg = out_tiles[kt][:, sl]
                        nc.vector.tensor_tensor(nt_seg, x_tiles[kt][:, sl],
                                                mu_b[hh][:, :], OP.subtract)
                    # -var = mu^2 - E[x^2] at the packed row
                    nc.vector.tensor_tensor(mup[r0:r0 + 1, :], mup[r0:r0 + 1, :],
                                            mup[r0:r0 + 1, :], OP.mult)
                    nc.vector.tensor_tensor(mup[r0:r0 + 1, :], mup[r0:r0 + 1, :],
                                            msqp[r0:r0 + 1, :], OP.subtract)
                    # rstd = rsqrt(var + eps) in one ACT op
                    raw_act(rstd[r0:r0 + 1, :], mup[r0:r0 + 1, :],
                            AF.Rsqrt, scale=-1.0,
                            bias=bias_sb[0:1, EPS_COL:EPS_COL + 1])
                for hh in range(2):
                    sl = slice(hh * QH, (hh + 1) * QH)
                    r0 = 32 * hh
                    rb = pps.tile([128, QH], FP32, tag="avps", bufs=2,
                                  name=f"rb{which}{hh}")
                    nc.tensor.matmul(rb[:, :], onesr[r0:r0 + 1, :],
                                     rstd[r0:r0 + 1, :], start=True, stop=True)
                    for kt in range(DKT):
                        nt_seg = out_tiles[kt][:, sl]
                        nc.vector.tensor_tensor(nt_seg, nt_seg, rb[:, :], OP.mult)
                        if not ln_trivial[which - 1]:
                            nc.scalar.activation(nt_seg, nt_seg, AF.Identity,
                                                 bias=bcol(LN_B[which] + kt),
                                                 scale=bcol(LN_G[which] + kt))
                return out_tiles

            def dense_proj(w_big, n_tiles, n_kt, tag):
                """Dense out^T: 5 x [128, T] bf16 tiles via full-height
                matmuls (the per-head-padded layout wastes 37.5% of the PE
                array on 80-row outputs; here every matmul fills 128 rows
                and the head split happens via SBUF->SBUF DMA in repart)."""
                outs = []
                for c in range(DKT):
                    qp = pps.tile([128, 2 * QH], FP32, tag="sps", bufs=3,
                                  name=f"dp{tag}{c}")
                    for hh in range(2):
                        for kt in range(n_kt):
                            nc.tensor.matmul(
                                qp[:, hh * QH:(hh + 1) * QH],
                                wsl(w_big, kt, D, c * 128, (c + 1) * 128),
                                n_tiles[kt][:, hh * QH:(hh + 1) * QH],
                                start=(kt == 0), stop=(kt == n_kt - 1))
                    qd = pqd.tile([128, T], BF16, tag="qd", bufs=3,
                                  name=f"qd{tag}{c}")
                    nc.vector.tensor_copy(qd[:], qp[:, :])
                    outs.append(qd)
                return outs

            def repart(dense_tiles, dst_tiles, col_off):
                """dense feature rows 80h:80h+80 -> dst_tiles[h][0:80]
                (per-head padded layout) via partition-crossing SBUF DMA."""
                for h in range(H):
                    c0, r0 = divmod(DH * h, 128)
                    n0 = min(128 - r0, DH)
                    nc.sync.dma_start(dst_tiles[h][0:n0, col_off:col_off + T],
                                      dense_tiles[c0][r0:r0 + n0, :])
                    if n0 < DH:
                        nc.sync.dma_start(
                            dst_tiles[h][n0:DH, col_off:col_off + T],
                            dense_tiles[c0 + 1][0:DH - n0, :])

            def repart_rev(aT_t, tag):
                """per-head attention out [0:80] -> 5 dense [128, T] tiles so
                the out-projection contracts 640 rows instead of 1024 padded."""
                ad = [pad.tile([128, T], BF16, tag="ad", bufs=5,
                               name=f"ad{tag}{c}") for c in range(DKT)]
                for h in range(H):
                    c0, r0 = divmod(DH * h, 128)
                    n0 = min(128 - r0, DH)
                    nc.sync.dma_start(ad[c0][r0:r0 + n0, :], aT_t[h][0:n0, :])
                    if n0 < DH:
                        nc.sync.dma_start(ad[c0 + 1][0:DH - n0, :],
                                          aT_t[h][n0:DH, :])
                return ad

            def v_proj(n_tiles, vt, n_kt, w_big, n_tok, tok_off):
                """token-major V tile, per-head 97-col slots: data cols 0:80,
                ones col at 96 so the AV denominator lands on PSUM partition
                96 (engine APs must start at partition 0/32/64/96)."""
                slots = vt[:, 0:H * VSLOT].rearrange("p (h c) -> p h c", c=VSLOT)
                nc.gpsimd.memset(slots[:, :, 80:96], 0.0)
                nc.gpsimd.memset(slots[:, :, 96:97], 1.0)
                vpp = pps.tile([128, 2 * QH], FP32, tag="sps", bufs=3, name="vpp")
                for half in range(2):
                    vp = vpp[:, half * QH:half * QH + 320]
                    for kt in range(n_kt):
                        nc.tensor.matmul(
                            vp[0:n_tok, :],
                            n_tiles[kt][:, tok_off:tok_off + n_tok],
                            wsl(w_big, kt, D, half * 320, (half + 1) * 320),
                            start=(kt == 0), stop=(kt == n_kt - 1))
                    dst = vt[:, half * 4 * VSLOT:(half + 1) * 4 * VSLOT].rearrange(
                        "p (h c) -> p h c", c=VSLOT)[0:n_tok, :, 0:80]
                    src = vp[0:n_tok, :].rearrange("p (h c) -> p h c", c=80)
                    nc.vector.tensor_copy(dst, src)

            def attention(qT_t, kT_t, v_t, n_keytiles, key_dim_last, aT_t, e_pool,
                          recip_on_act=False, delay_normalize=False):
                """S^T -> exp -> AV; attention output is evicted unnormalized
                and the 16 per-(head, q-half) denominators are batched into
                four 32-row-aligned tiles so just four reciprocals run.
                recip_on_act uses one batched ACT Reciprocal per tile instead
                of the DVE divide pipeline; delay_normalize postpones all
                normalizes past the last exp so the ACT table is swapped
                exactly once."""
                den_t = {}
                denr_t = {}
                GRP = 3  # matmul APs may only start at partition 0/32/64

                def dslot(p):
                    return p // GRP, 32 * (p % GRP)

                def emit_group_normalize(t):
                    """reciprocal of den tile t + normalize its pairs."""
                    dr = prcb.tile([128, QH], F32R, tag="denr", bufs=2,
                                   name=f"denr{t}")
                    if recip_on_act:
                        raw_act(dr[:], den_t[t][:], AF.Reciprocal)
                    else:
                        with nc.allow_low_precision(reason="fp32r denom rounding"):
                            nc.vector.reciprocal(dr[:], den_t[t][:])
                    denr_t[t] = dr
                    for p in range(GRP * t, min(GRP * t + GRP, n_pairs)):
                        h, hh = p // 2, p % 2
                        _, drow = dslot(p)
                        rcb = pps.tile([128, QH], FP32, tag="avps", bufs=2,
                                       name=f"rcb{h}{hh}")
                        nc.tensor.matmul(
                            rcb[0:DH, :], onesr[drow:drow + 1, 0:DH],
                            dr[drow:drow + 1, :], start=True, stop=True)
                        seg = aT_t[h][0:DH, hh * QH:(hh + 1) * QH]
                        nc.vector.tensor_tensor(seg, seg, rcb[0:DH, :], OP.mult)
                npairs = (n_keytiles + 1) // 2
                n_pairs = 2 * H
                for h in range(H):
                    at = aT_t[h]
                    # (rows 80:128 are never read -- the dense out-proj takes
                    # only rows 0:80 through repart_rev -- so no pad memset)
                    for hh in range(2):
                        p = h * 2 + hh
                        avp = pps.tile([128, QH], FP32, tag="avps", bufs=2,
                                       name=f"av{h}{hh}")
                        # two score tiles share one 2-bank PSUM tile so a
                        # single exp covers both (halves the ACT op count);
                        # pipelined TWO pairs ahead of the AV consumers so the
                        # combo-tail AV never waits on the freshest exp
                        ets = {}
                        for pt in range(npairs + 2):
                            if pt < npairs:
                                kts = [kt for kt in (2 * pt, 2 * pt + 1)
                                       if kt < n_keytiles]
                                spp = pps.tile([128, 2 * QH], FP32, tag="sps",
                                               bufs=3, name=f"s{h}{hh}{pt}")
                                klens = []
                                for j, kt in enumerate(kts):
                                    klen = (key_dim_last
                                            if kt == n_keytiles - 1 else 128)
                                    klens.append(klen)
                                    nc.tensor.matmul(
                                        spp[0:klen, j * QH:(j + 1) * QH],
                                        kT_t[h][0:DH, kt * 128:kt * 128 + klen],
                                        qT_t[h][0:DH, hh * QH:(hh + 1) * QH],
                                        start=True, stop=True)
                                et = e_pool.tile([128, 2 * QH], BF16, tag="E",
                                                 name=f"e{h}{hh}{pt}")
                                if len(kts) == 2 and klens[0] == klens[1]:
                                    nc.scalar.activation(
                                        et[0:klens[0], :], spp[0:klens[0], :],
                                        AF.Exp, scale=scale)
                                else:
                                    for j, kt in enumerate(kts):
                                        nc.scalar.activation(
                                            et[0:klens[j], j * QH:(j + 1) * QH],
                                            spp[0:klens[j], j * QH:(j + 1) * QH],
                                            AF.Exp, scale=scale)
                                ets[pt] = (et, kts, klens)
                            if pt >= 2:
                                pet, pkts, pklens = ets.pop(pt - 2)
                                for j, kt in enumerate(pkts):
                                    nc.tensor.matmul(
                                        avp[0:VSLOT, :],
                                        v_t[kt][0:pklens[j], h * VSLOT:(h + 1) * VSLOT],
                                        pet[0:pklens[j], j * QH:(j + 1) * QH],
                                        start=(kt == 0), stop=(kt == n_keytiles - 1))
                        # unnormalized evict (frees the PSUM bank) + denom stash
                        nc.vector.tensor_copy(at[0:DH, hh * QH:(hh + 1) * QH],
                                              avp[0:DH, :])
                        dt_i, drow = dslot(p)
                        if dt_i not in den_t:
                            dn = prcb.tile([128, QH], BF16, tag="den", bufs=6,
                                           name=f"den{dt_i}")
                            nc.gpsimd.memset(dn[:], 1.0)
                            den_t[dt_i] = dn
                        nc.vector.tensor_copy(
                            den_t[dt_i][drow:drow + 1, :], avp[96:97, :])
                        if not delay_normalize and (
                                p == GRP * dt_i + GRP - 1 or p == n_pairs - 1):
                            emit_group_normalize(dt_i)
                if delay_normalize:
                    for t in sorted(den_t):
                        emit_group_normalize(t)

            def out_proj(w_big, ad_t, res_t, bias_off):
                """res += a @ o + bias (in-place residual update), contracting
                5 dense feature tiles; token-half-major so the next LN's
                half-0 stats can start while half 1 is still projecting."""
                for hh in range(2):
                    sl = slice(hh * QH, (hh + 1) * QH)
                    for m in range(DKT):
                        op_ = pps.tile([128, QH], FP32, tag="avps", bufs=2,
                                       name=f"op{m}{hh}")
                        for kt in range(DKT):
                            nc.tensor.matmul(
                                op_[:, :],
                                wsl(w_big, kt, D, m * 128, (m + 1) * 128),
                                ad_t[kt][:, sl],
                                start=(kt == 0), stop=(kt == DKT - 1))
                        nc.vector.scalar_tensor_tensor(
                            res_t[m][:, sl], op_[:, :], bcol(bias_off + m),
                            res_t[m][:, sl], OP.add, OP.add)

            def load_frame(dname, tag, pool, dtype, bufs=1):
                """One [128, 5*T] tile per frame, single DMA descriptor."""
                ft = pool.tile([128, DKT * T], dtype, tag=tag, name=dname,
                               bufs=bufs)
                dst = ft[:].rearrange("p (k c) -> p k c", c=T)
                src = dram[dname].rearrange("(k p) c -> p k c", p=128)
                nc.sync.dma_start(dst, src)
                return [ft[:, kt * T:(kt + 1) * T] for kt in range(DKT)]

            # residual stream (feature-major, f32), one DMA descriptor
            res_tiles = load_frame("hsT_q", "res", pres, F32R)

            with ExitStack() as ctx_abcd:
                pqT = ctx_abcd.enter_context(tc.tile_pool(name="qT", bufs=8))
                paT = ctx_abcd.enter_context(tc.tile_pool(name="aT", bufs=8))
                penc = ctx_abcd.enter_context(tc.tile_pool(name="enc", bufs=1))
                pk2 = ctx_abcd.enter_context(tc.tile_pool(name="k2T", bufs=8))
                pV2 = ctx_abcd.enter_context(tc.tile_pool(name="V2", bufs=1))
                pwp = ctx_abcd.enter_context(tc.tile_pool(name="wp", bufs=1))
                pqd = ctx_abcd.enter_context(tc.tile_pool(name="qd", bufs=3))

                # ---------- phase A: LN1 + QKV projections ----------
                with ExitStack() as ctx_b:
                    pfr = ctx_b.enter_context(tc.tile_pool(name="fr", bufs=2))
                    pkT = ctx_b.enter_context(tc.tile_pool(name="kT", bufs=8))
                    pV = ctx_b.enter_context(tc.tile_pool(name="V", bufs=16))
                    pE = ctx_b.enter_context(tc.tile_pool(name="E", bufs=3))

                    kT_tiles = [pkT.tile([128, 2 * T], BF16, tag="kT", name=f"kT_{h}")
                                for h in range(H)]
                    v_tiles = [pV.tile([128, H * VSLOT], BF16, tag="V", name=f"v_{i}")
                               for i in range(2 * TT)]

                    n_q = emit_ln(res_tiles, 1, "n")
                    fr0_tiles = load_frame("hsT_first", "fr", pfr, BF16, bufs=2)
                    emit_ln(fr0_tiles, 1)  # in place, overlaps Q proj
                    q1_sb = load_w_big("q1", DKT, D, "w", pw)
                    # issue the cross-attn input/weight fetches early; their
                    # projections are emitted after the fi loop
                    enc_big = penc.tile([128, CKT * CTXP], BF16, tag="enc",
                                        name="enc")
                    enc_dst = enc_big[:].rearrange("p (k c) -> p k c", c=CTXP)
                    enc_src = dram["encT"].rearrange("(k p) c -> p k c", p=128)
                    nc.sync.dma_start(enc_dst, enc_src)
                    k2_sb = load_w_big("k2", CKT, D, "w6", pw, bufs=1)
                    qT_tiles = [pqT.tile([128, T], BF16, tag="qT", name=f"qT_{h}")
                                for h in range(H)]
                    repart(dense_proj(q1_sb, n_q, DKT, "q"), qT_tiles, 0)

                    k1_sb = load_w_big("k1", DKT, D, "w", pw)
                    v1_sb = load_w_big("v1", DKT, D, "w", pw)
                    fr1_tiles = load_frame("hsT_former", "fr", pfr, BF16, bufs=2)
                    emit_ln(fr1_tiles, 1)  # in place, overlaps K/V of frame 0
                    for fi, fr_n in enumerate((fr0_tiles, fr1_tiles)):
                        repart(dense_proj(k1_sb, fr_n, DKT, f"k{fi}"),
                               kT_tiles, fi * T)
                        for tt in range(TT):
                            v_proj(fr_n, v_tiles[fi * TT + tt], DKT, v1_sb,
                                   128, tt * 128)

                    # cross-attn K/V: no dependency on attn1 -- build early
                    enc_tiles = [enc_big[:, kt * CTXP:(kt + 1) * CTXP]
                                 for kt in range(CKT)]
                    k2T_tiles = [pk2.tile([128, CTXP], BF16, tag="k2T",
                                          name=f"k2T_{h}") for h in range(H)]
                    for h in range(H):
                        kp = pps.tile([128, CTXP], FP32, tag="avps", bufs=2,
                                      name=f"k2p{h}")
                        for kt in range(CKT):
                            nc.tensor.matmul(kp[0:DH, :],
                                             wsl(k2_sb, kt, D, h * DH, (h + 1) * DH),
                                             enc_tiles[kt],
                                             start=(kt == 0), stop=(kt == CKT - 1))
                        nc.vector.tensor_copy(k2T_tiles[h][0:DH, :], kp[0:DH, :])
                    v2_sb = load_w_big("v2", CKT, D, "w6", pw, bufs=1)
                    v2_t = pV2.tile([128, H * VSLOT], BF16, tag="V2", name="v2t")
                    v_proj(enc_tiles, v2_t, CKT, v2_sb, CTX, 0)
                    # prefetch the o1 out-proj weights behind the attention PE stream
                    o1_sb = load_w_big("o1", DKT, D, "wp", pwp, bufs=1)

                    # ---------- phase B: sparse-causal attention ----------
                    aT_tiles = [paT.tile([128, T], BF16, tag="aT", name=f"aT_{h}")
                                for h in range(H)]
                    attention(qT_tiles, kT_tiles, v_tiles, 2 * TT, 128, aT_tiles, pE)

                # ---------- phases C+D: o1 + residual, cross attention ----------
                with ExitStack() as ctx_d:
                    pad = ctx_d.enter_context(tc.tile_pool(name="ad", bufs=5))
                    pE2 = ctx_d.enter_context(tc.tile_pool(name="E2", bufs=3))

                    out_proj(o1_sb, repart_rev(aT_tiles, "a1"), res_tiles, OB1)

                    o2_sb = load_w_big("o2", DKT, D, "wp", pwp, bufs=1)
                    n2 = emit_ln(res_tiles, 2, "n")
                    q2_sb = load_w_big("q2", DKT, D, "w", pw)
                    q2T_tiles = [pqT.tile([128, T], BF16, tag="qT", name=f"q2T_{h}")
                                 for h in range(H)]
                    repart(dense_proj(q2_sb, n2, DKT, "q2"), q2T_tiles, 0)

                    a2T_tiles = [paT.tile([128, T], BF16, tag="aT", name=f"a2T_{h}")
                                 for h in range(H)]
                    attention(q2T_tiles, k2T_tiles, [v2_t], 1, CTX, a2T_tiles, pE2,
                              recip_on_act=True, delay_normalize=True)
                    out_proj(o2_sb, repart_rev(a2T_tiles, "a2"), res_tiles, OB2)

            # ---------- phase E: GEGLU feed-forward ----------
            with ExitStack() as ctx_e:
                pG = ctx_e.enter_context(tc.tile_pool(name="gT", bufs=20))
                pgl = ctx_e.enter_context(tc.tile_pool(name="gl", bufs=3))
                pff1 = ctx_e.enter_context(tc.tile_pool(name="ff1w", bufs=40))
                pff2 = ctx_e.enter_context(tc.tile_pool(name="ff2w", bufs=1))

                # issue every ff weight DMA up front so the fetch overlaps the
                # cross-attention tail instead of trickling in per row-tile
                fxg = []
                for mi in range(2 * FMT):
                    fw = pff1.tile([128, D], BF16, tag="ff1w", name=f"fw{mi}")
                    fw_dst = fw[:].rearrange("p (k c) -> p k c", c=128)
                    fw_src = dram["ff1b"][mi].rearrange("(k p) c -> p k c", p=128)
                    nc.sync.dma_start(fw_dst, fw_src)
                    fxg.append(fw)
                ff2_sb = load_w_big("ff2", FMT, D, "ff2w", pff2, bufs=1)

                n3 = emit_ln(res_tiles, 3, "n")
                # token-half-major: ff1+ff2 for half 0 only wait on LN3's
                # half-0 stats, overlapping the cross-attention tail
                gT_tiles = []
                for hh in range(2):
                    for mi in range(FMT):
                        fx, fg = fxg[mi], fxg[FMT + mi]
                        if hh == 0:
                            gT_tiles.append(pG.tile([128, T], BF16, tag="gT",
                                                    name=f"gT_{mi}"))
                        gt = gT_tiles[mi]
                        xgp = pps.tile([128, 2 * QH], FP32, tag="sps", bufs=3,
                                       name=f"xgp{mi}{hh}")
                        xp = xgp[:, 0:QH]
                        gp = xgp[:, QH:2 * QH]
                        for kt in range(DKT):
                            nc.tensor.matmul(
                                xp[:, :], fx[:, kt * 128:(kt + 1) * 128],
                                n3[kt][:, hh * QH:(hh + 1) * QH],
                                start=(kt == 0), stop=(kt == DKT - 1))
                        for kt in range(DKT):
                            nc.tensor.matmul(
                                gp[:, :], fg[:, kt * 128:(kt + 1) * 128],
                                n3[kt][:, hh * QH:(hh + 1) * QH],
                                start=(kt == 0), stop=(kt == DKT - 1))
                        gl = pgl.tile([128, QH], BF16, tag="gl", name=f"gl{mi}{hh}")
                        nc.scalar.activation(gl[:], gp[:, :], GELU_AF or AF.Gelu,
                                             bias=bcol(FBG + mi), scale=1.0)
                        nc.vector.scalar_tensor_tensor(
                            gt[:, hh * QH:(hh + 1) * QH], xp[:, :], bcol(FBX + mi),
                            gl[:], OP.add, OP.mult)

                    sl = slice(hh * QH, (hh + 1) * QH)
                    for m in range(DKT):
                        fp = pps.tile([128, QH], FP32, tag="avps", bufs=2,
                                      name=f"fp{m}{hh}")
                        for kt in range(FMT):
                            nc.tensor.matmul(
                                fp[:, :],
                                wsl(ff2_sb, kt, D, m * 128, (m + 1) * 128),
                                gT_tiles[kt][:, sl],
                                start=(kt == 0), stop=(kt == FMT - 1))
                        nc.vector.scalar_tensor_tensor(
                            res_tiles[m][:, sl], fp[:, :], bcol(FB2 + m),
                            res_tiles[m][:, sl], OP.add, OP.add)
                        # stream the finished [chunk, half] straight out
                        nc.sync.dma_start(
                            out_dram[m * 128:(m + 1) * 128, sl],
                            res_tiles[m][:, sl])

    nc.compile()
    return nc


def _install_ntff_shim():
    """Register the axon NTFF profile hook (profiling only; this container's
    antenv lacks the axon_hooks shim module)."""
    import types
    if "antenv.axon_hooks" in sys.modules:
        return
    mod = types.ModuleType("antenv.axon_hooks")
    mod._hook = None
    mod.set_axon_ntff_profile_hook = lambda h: setattr(mod, "_hook", h)
    mod.get_axon_ntff_profile_hook = lambda: mod._hook
    sys.modules["antenv.axon_hooks"] = mod
    try:
        from trn_agent_boot.trn_boot import _ntff_profile_via_ctypes
        mod._hook = _ntff_profile_via_ctypes("/opt/axon/libaxon_pjrt.so")
    except Exception:
        pass


_PROGRAM_CACHE = {}


def _get_program(ln_trivial):
    key = (tuple(ln_trivial), GELU_AF)
    if key not in _PROGRAM_CACHE:
        _PROGRAM_CACHE[key] = build_program(ln_trivial)
    return _PROGRAM_CACHE[key]


def _pad_heads(w):
    """[640, 640] head rows -> [1024, 640] padded to 128/head."""
    out = np.zeros((H * 128, D), np.float32)
    for h in range(H):
        out[h * 128:h * 128 + DH] = w[h * DH:(h + 1) * DH]
    return out


def _bias_cols(vec, n):
    return np.ascontiguousarray(vec.reshape(n, 128).T)


def _bf(a):
    return np.ascontiguousarray(np.asarray(a, np.float32)).astype(ml_dtypes.bfloat16)


def kernel(**inputs):
    hs = np.ascontiguousarray(inputs["hidden_states"], np.float32)
    enc = np.ascontiguousarray(inputs["encoder_hidden_states"], np.float32)
    f = int(inputs["video_length"])
    BF = hs.shape[0]
    assert BF == N_CORES and hs.shape[1:] == (T, D)

    ln_trivial = tuple(
        bool(np.all(inputs[f"n{i}_g"] == 1.0) and np.all(inputs[f"n{i}_b"] == 0.0))
        for i in (1, 2, 3))
    nc = _get_program(ln_trivial)

    biases = np.zeros((128, NB), np.float32)
    biases[:, EPS_COL] = LN_EPS
    biases[:, OB1:OB1 + 5] = _bias_cols(inputs["o1_b"].astype(np.float32), 5)
    biases[:, OB2:OB2 + 5] = _bias_cols(inputs["o2_b"].astype(np.float32), 5)
    biases[:, FB2:FB2 + 5] = _bias_cols(inputs["ff2_b"].astype(np.float32), 5)
    ff1_b = inputs["ff1_b"].astype(np.float32)
    biases[:, FBX:FBX + FMT] = _bias_cols(ff1_b[:DFF], FMT)
    biases[:, FBG:FBG + FMT] = _bias_cols(ff1_b[DFF:], FMT)
    for i in (1, 2, 3):
        biases[:, LN_G[i]:LN_G[i] + 5] = _bias_cols(inputs[f"n{i}_g"].astype(np.float32), 5)
        biases[:, LN_B[i]:LN_B[i] + 5] = _bias_cols(inputs[f"n{i}_b"].astype(np.float32), 5)

    ff1 = inputs["ff1"].astype(np.float32)  # [640, 5120]
    ff1b = np.ascontiguousarray(
        ff1.reshape(DKT, 128, 2 * FMT, 128).transpose(2, 0, 1, 3).reshape(2 * FMT, D, 128))

    common = {
        "q1": _bf(inputs["q1"]),
        "k1": _bf(inputs["k1"]),
        "v1": _bf(inputs["v1"]),
        "q2": _bf(inputs["q2"]),
        "k2": _bf(inputs["k2"]),
        "v2": _bf(inputs["v2"]),
        "o1": _bf(inputs["o1"]),
        "o2": _bf(inputs["o2"]),
        "ff1b": ff1b.astype(ml_dtypes.bfloat16),
        "ff2": _bf(inputs["ff2"]),
        "biases": biases,
    }

    hsT = np.ascontiguousarray(hs.transpose(0, 2, 1))      # [BF, 640, 1024]
    hsTb = hsT.astype(ml_dtypes.bfloat16)
    encT = np.zeros((BF, CROSS, CTXP), np.float32)         # ctx padded 77 -> 80
    encT[:, :, :CTX] = enc.transpose(0, 2, 1)
    encTb = encT.astype(ml_dtypes.bfloat16)
    in_maps = []
    for g in range(BF):
        bi, fi = divmod(g, f)
        first = bi * f
        former = bi * f + max(fi - 1, 0)
        in_maps.append({
            **common,
            "hsT_q": hsT[g],
            "hsT_first": hsTb[first],
            "hsT_former": hsTb[former],
            "encT": encTb[g],
        })

    want_trace = bool(int(os.environ.get("KERNEL_TRACE", "0")))
    if want_trace:
        _install_ntff_shim()
    res = run_bass_kernel_spmd(nc, in_maps, core_ids=list(range(N_CORES)),
                               trace=want_trace)
    kernel.last_results = res
    out = np.stack([res.results[g]["outT"].T for g in range(BF)])
    return np.ascontiguousarray(out.astype(inputs["hidden_states"].dtype))
